# revision 25
# baseline (speedup 1.0000x reference)
"""MoE grouped linear (DMoELinear) on 8 Trainium2 NeuronCores.

Expert-parallel sharding: tokens are sorted by expert id, so expert e's
tokens form one contiguous slice. Core e receives expert e's tokens
(padded to a uniform capacity C = max group size, so all cores run one
SPMD NEFF), expert e's weight and bias, and computes
    yT_e = (x_e @ W_e.T).T.bf16 + b_e.bf16
with the weight block as the stationary matmul operand and tokens as
the moving free dim.

v9 schedule (trace-driven). Measured constraints: each dma_start costs
~0.6us of issuing-engine time; each engine rotates 4 DMA queues (issue
N+4 blocks on N's completion); per-ring throughput under 8-core HBM
contention is ~175-230GB/s. The 3.2MB trickle stream (x + w0/w1) is
the binding constraint for the first ~17us, so it rides few, large
transfers interleaved by first-use deadline; w2 follows split in
halves (db2 runs k-major so it only needs w2k0 at the trickle's
drain), then w3 and the 2-db packs with multi-us slack. (int8 weight
shipping + on-chip upconvert was tried and abandoned: the only fast
int8->bf16 path is ACTIVATE-Copy on the ACT engine (~115G elem/s; the
DVE/Pool tensor_scalar path measures ~9G under load) and it cannot
pace the trickle, while the late tensors it can pace were never
deadline-bound.)

PE schedule: 32 warm matmuls flip the HAM clock gate while the first
tiles land (the real stream gates the start at ~11.5 anyway); db0/db1
interleave k-major (trickle, DMA-paced); db2 runs k-major; dbs 3..14
run chunk-major (c0's whole k-loop first, evicted while c1/c2 —
interleaved so the narrow chunk's LDWEIGHTS hide under 512-wide
drains — still compute) so PSUM banks retire mid-db; db15 splits its
middle 512 region into 256+128+128 pieces with per-piece PSUM tiles
(shared psum tiles serialize readers in the dep tracker) so after the
last matmul only two parallel 128-wide evictions and one 64KB DMA
remain. Bias adds fuse into the PSUM evictions (ACT/DVE alternating).
"""

import numpy as np
import ml_dtypes

N_TOK, D_IN, D_OUT, N_EXP = 8192, 1024, 2048, 8
N_CORES = 8
P = 128
NFREE = 512  # max matmul moving free dim (one PSUM bank of f32)

BF16 = ml_dtypes.bfloat16

_nc_cache: dict[int, object] = {}


def _chunks(C):
    out = []
    off = 0
    while off < C:
        cw = min(NFREE, C - off)
        out.append((off, cw))
        off += cw
    return out


def _build_bass(C: int):
    """Emit the per-core Bass/Tile kernel for token capacity C."""
    import concourse.bass as bass  # noqa: F401  (registers engines)
    import concourse.mybir as mybir
    import concourse.tile as tile
    from concourse import bacc

    dt = mybir.dt
    KT = D_IN // P      # 8 contraction tiles
    DB = D_OUT // P     # 16 output-row blocks
    KW = KT * P         # columns per db block in the flat weight (1024)
    chunks = _chunks(C)
    chunk_of_db = {db: chunks for db in range(DB)}

    nc = bacc.Bacc("TRN2", target_bir_lowering=False)

    # x partition-flat: row p, col ki*C + c  =  x[token c, ki*128+p]
    xf_d = nc.dram_tensor("xf", [P, KT * C], dt.bfloat16, kind="ExternalInput")
    # flat weights: row p, col db*1024 + kt*128 + d  (lhsT slices are
    # contiguous 128-col blocks; multi-db packs are contiguous too).
    wf_d = nc.dram_tensor("wf", [P, DB * KW], dt.bfloat16, kind="ExternalInput")
    bias_d = nc.dram_tensor("biasp", [P, DB], dt.float32, kind="ExternalInput")
    y_d = nc.dram_tensor("yT", [D_OUT, C], dt.bfloat16, kind="ExternalOutput")

    with tile.TileContext(nc) as tc:
        with (
            tc.tile_pool(name="persist", bufs=1) as ppool,
            tc.tile_pool(name="yout", bufs=4) as ypool,
            tc.tile_pool(name="psum", bufs=8, space="PSUM") as pspool,
        ):
            x_tiles = [
                ppool.tile([P, C], dt.bfloat16, name=f"x{ki}", tag=f"x{ki}")
                for ki in range(KT)
            ]

            def x_sl(ki, off, cw):
                return x_tiles[ki][:, off:off + cw]

            w_s = [
                ppool.tile([P, KW], dt.bfloat16, name=f"w{db}", tag=f"w{db}")
                for db in range(4)
            ]
            packs = [
                ppool.tile([P, 2 * KW], dt.bfloat16, name=f"wp{g}", tag=f"wp{g}")
                for g in range(2, 8)
            ]
            bt = ppool.tile([P, DB], dt.float32, name="bias", tag="bias")

            def lhsT(db, ki):
                if db < 4:
                    return w_s[db][:, ki * P:(ki + 1) * P]
                g = db // 2
                off = (db - 2 * g) * KW + ki * P
                return packs[g - 2][:, off:off + P]

            # ── DMA schedule: two HWDGE rings ────────────────────────
            A, B = nc.sync, nc.scalar

            def xdma(ki, eng):
                eng.dma_start(x_tiles[ki][:], xf_d[:, ki * C:(ki + 1) * C])

            # w0/w1 in k-halves: the first real matmul is gated by
            # w0k0, so a 256KB half lands ~1.3us earlier than the
            # whole slab; the k4-7 halves are only needed 4 steps in.
            xdma(1, A)
            xdma(0, B)
            A.dma_start(w_s[0][:, 0:4 * P], wf_d[:, 0:4 * P])
            B.dma_start(w_s[1][:, 0:4 * P], wf_d[:, KW:KW + 4 * P])
            xdma(3, A)
            xdma(2, B)
            A.dma_start(w_s[0][:, 4 * P:KW], wf_d[:, 4 * P:KW])
            B.dma_start(w_s[1][:, 4 * P:KW], wf_d[:, KW + 4 * P:2 * KW])
            xdma(5, A)
            xdma(4, B)
            xdma(7, A)
            xdma(6, B)
            B.dma_start(bt[:], bias_d[:])
            A.dma_start(w_s[2][:, 0:4 * P], wf_d[:, 2 * KW:2 * KW + 4 * P])
            A.dma_start(w_s[2][:, 4 * P:8 * P], wf_d[:, 2 * KW + 4 * P:3 * KW])
            B.dma_start(w_s[3][:], wf_d[:, 3 * KW:4 * KW])
            for g in range(2, 8):
                eng = B if g % 2 == 0 else A
                eng.dma_start(packs[g - 2][:], wf_d[:, 2 * g * KW:(2 * g + 2) * KW])

            # ── PE warmup: flip the HAM clock gate (~3.5us of activity)
            # while the first DMAs land. The first real matmul is gated
            # by the w0 slab landing ~11.5 anyway.
            warm = ppool.tile([P, P], dt.bfloat16, name="warm", tag="warm")
            nc.vector.memset(warm[:], 0.0)
            wps = pspool.tile([P, P], dt.float32, name="wps", tag="ps")

            def warm_mm(n):
                for _ in range(n):
                    nc.tensor.matmul(wps[:], warm[:], warm[:], start=True, stop=True)

            warm_mm(32)

            all_psums = {}

            def alloc_chunk(db, j):
                _, cw = chunk_of_db[db][j]
                return pspool.tile([P, cw], dt.float32, name=f"ps{db}_{j}", tag="ps")

            def alloc_psums(db, chunks_j=None):
                js = chunks_j or range(len(chunk_of_db[db]))
                cur = all_psums.setdefault(db, {})
                for j in js:
                    cur[j] = alloc_chunk(db, j)

            def emit_mm(db, ki, j):
                off, cw = chunk_of_db[db][j]
                nc.tensor.matmul(
                    all_psums[db][j][:, :cw],
                    lhsT(db, ki),
                    x_sl(ki, off, cw),
                    start=(ki == 0),
                    stop=(ki == KT - 1),
                )

            def emit_mms(db, ki, chunks_j=None):
                for j in chunks_j or range(len(chunk_of_db[db])):
                    emit_mm(db, ki, j)

            ep = 0
            ysbs = {}

            def new_ysb(db):
                ysbs[db] = ypool.tile([P, C], dt.bfloat16, name="ysb", tag="ysb")
                return ysbs[db]

            def evict_chunk(db, j, ysb):
                nonlocal ep
                off, cw = chunk_of_db[db][j]
                bias_col = bt[:, db:db + 1]
                if ep % 2 == 0:
                    nc.scalar.add(ysb[:, off:off + cw], all_psums[db][j][:, :cw], bias_col)
                else:
                    nc.vector.tensor_scalar_add(
                        ysb[:, off:off + cw], all_psums[db][j][:, :cw], bias_col
                    )
                ep += 1

            def evict(db):
                ysb = new_ysb(db)
                for j in range(len(chunk_of_db[db])):
                    evict_chunk(db, j, ysb)
                return ysb

            def ydma(db, ysb):
                eng = nc.sync if db % 2 == 0 else nc.scalar
                eng.dma_start(y_d[db * P:(db + 1) * P, :], ysb[:])

            # ── Trickle phase ────────────────────────────────────────
            # db0/db1 interleaved by k-step; db1 one step behind so
            # db0's k7 chunks finish (and their PSUM banks evict) while
            # db1's tail runs.
            STAG = 1
            alloc_psums(0)
            alloc_psums(1)
            for step in range(KT + STAG):
                if step < KT:
                    emit_mms(0, step)
                if step >= STAG:
                    emit_mms(1, step - STAG)
            ydma(0, evict(0))
            ydma(1, evict(1))

            # db2 stays k-major: it starts right at the trickle's drain
            # and k-major only needs w2k0 by then (chunk-major would
            # need all of w2, which is still streaming).
            alloc_psums(2)
            for ki in range(KT):
                emit_mms(2, ki)
            ydma(2, evict(2))

            # ── dbs 3..14: chunk-major ───────────────────────────────
            # c0 runs its whole k-loop first and evicts while c1/c2
            # (interleaved so the narrow chunk's LDWEIGHTS hide under
            # the 512-wide drains) are still computing. PSUM banks
            # retire mid-db instead of piling up at db boundaries.
            for db in range(3, DB - 1):
                alloc_psums(db)
                ncks = len(chunk_of_db[db])
                for ki in range(KT):
                    emit_mm(db, ki, 0)
                ysb = new_ysb(db)
                evict_chunk(db, 0, ysb)
                for ki in range(KT):
                    for j in range(1, ncks):
                        emit_mm(db, ki, j)
                for j in range(1, ncks):
                    evict_chunk(db, j, ysb)
                if db == DB - 2:
                    # per-chunk DMAs on both rings so the tail pipelines
                    for j, (off, cw) in enumerate(chunk_of_db[db]):
                        eng = nc.sync if (db + j) % 2 == 0 else nc.scalar
                        eng.dma_start(
                            y_d[db * P:(db + 1) * P, off:off + cw],
                            ysb[:, off:off + cw],
                        )
                else:
                    ydma(db, ysb)

            # ── db15: ordered so the drain is minimal ────────────────
            # (c0, tail-chunk) interleaved first — both evicted and
            # DMA'd while the middle 512 region computes as 256+128+128
            # pieces with per-piece PSUM tiles. After the last matmul
            # only two parallel 128-wide evictions and one 64KB DMA on
            # the sync ring remain (c1a's 256KB rides the scalar ring
            # so the final piece doesn't queue behind it).
            db = DB - 1
            cks = chunk_of_db[db]
            row0 = db * P
            bias_col = bt[:, db:db + 1]
            if len(cks) == 3 and cks[1][1] == NFREE:
                (o0, cw0), (o1, cw1), (o2, cw2) = cks
                h = cw1 // 2
                hh = h // 2
                sub = [(o0, cw0), (o2, cw2), (o1, h),
                       (o1 + h, hh), (o1 + h + hh, cw1 - h - hh)]
                ps = {j: pspool.tile([P, cw], dt.float32, name=f"ps15_{j}", tag="ps")
                      for j, (off, cw) in enumerate(sub)}
                ysb = new_ysb(db)

                def mm15(j, ki):
                    off, cw = sub[j]
                    nc.tensor.matmul(
                        ps[j][:, :cw], lhsT(db, ki), x_sl(ki, off, cw),
                        start=(ki == 0), stop=(ki == KT - 1),
                    )

                for ki in range(KT):
                    mm15(0, ki)
                    mm15(1, ki)
                nc.scalar.add(ysb[:, o0:o0 + cw0], ps[0][:, :cw0], bias_col)
                nc.vector.tensor_scalar_add(
                    ysb[:, o2:o2 + cw2], ps[1][:, :cw2], bias_col
                )
                nc.sync.dma_start(y_d[row0:row0 + P, o0:o0 + cw0],
                                  ysb[:, o0:o0 + cw0])
                nc.scalar.dma_start(y_d[row0:row0 + P, o2:o2 + cw2],
                                    ysb[:, o2:o2 + cw2])
                for ki in range(KT):
                    mm15(2, ki)
                nc.vector.tensor_scalar_add(
                    ysb[:, o1:o1 + h], ps[2][:, :h], bias_col
                )
                nc.scalar.dma_start(y_d[row0:row0 + P, o1:o1 + h],
                                    ysb[:, o1:o1 + h])
                for ki in range(KT):
                    mm15(3, ki)
                    mm15(4, ki)
                o3, cw3 = sub[3]
                o4, cw4 = sub[4]
                nc.scalar.add(ysb[:, o3:o3 + cw3], ps[3][:, :cw3], bias_col)
                nc.vector.tensor_scalar_add(
                    ysb[:, o4:o4 + cw4], ps[4][:, :cw4], bias_col
                )
                nc.sync.dma_start(y_d[row0:row0 + P, o3:o3 + cw3 + cw4],
                                  ysb[:, o3:o3 + cw3 + cw4])
            else:
                # generic fallback (different C): plain chunk-major
                alloc_psums(db)
                for ki in range(KT):
                    emit_mm(db, ki, 0)
                ysb = new_ysb(db)
                evict_chunk(db, 0, ysb)
                for ki in range(KT):
                    for j in range(1, len(cks)):
                        emit_mm(db, ki, j)
                for j in range(1, len(cks)):
                    evict_chunk(db, j, ysb)
                for j, (off, cw) in enumerate(cks):
                    eng = nc.sync if j % 2 == 0 else nc.scalar
                    eng.dma_start(
                        y_d[row0:row0 + P, off:off + cw], ysb[:, off:off + cw]
                    )

    nc.compile()
    return nc


def _run_spmd(in_maps, C, trace=False, trace_cores=None):
    from concourse.bass_utils import run_bass_kernel_spmd

    nc = _nc_cache.get(C)
    if nc is None:
        nc = _build_bass(C)
        _nc_cache[C] = nc
    return run_bass_kernel_spmd(
        nc,
        in_maps,
        core_ids=list(range(N_CORES)),
        trace=trace,
        trace_cores=trace_cores,
    )


def _prepare(x, weight, bias, ids_sorted):
    """Host-side routing: returns (in_maps, C, counts, starts)."""
    x = np.asarray(x)
    weight = np.asarray(weight)
    bias = np.asarray(bias)
    ids = np.asarray(ids_sorted)

    counts = np.bincount(ids, minlength=N_EXP).astype(np.int64)
    starts = np.zeros(N_EXP, dtype=np.int64)
    starts[1:] = np.cumsum(counts)[:-1]
    C = max(int(counts.max()), 2)
    C += C % 2

    KT = D_IN // P
    DB = D_OUT // P
    xb = x.astype(BF16)
    in_maps = []
    for e in range(N_EXP):
        n_e = int(counts[e])
        xeT = np.zeros((D_IN, C), dtype=BF16)
        if n_e:
            xeT[:, :n_e] = xb[starts[e]:starts[e] + n_e].T
        # partition-flat x: row p, col ki*C + c = x[token c, ki*128+p]
        xf = np.ascontiguousarray(
            xeT.reshape(KT, P, C).transpose(1, 0, 2)
        ).reshape(P, KT * C)
        # flat weight: row p, col db*1024 + kt*128 + d  = W_e[db*128+d, kt*128+p]
        weT = weight[e].T.astype(BF16)  # [d_in, d_out]
        wf = np.ascontiguousarray(
            weT.reshape(KT, P, DB, P).transpose(1, 2, 0, 3)
        ).reshape(P, DB * KT * P)
        bp = np.ascontiguousarray(
            bias[e].astype(BF16).astype(np.float32).reshape(DB, P).T
        )
        in_maps.append({"xf": xf, "wf": wf, "biasp": bp})
    return in_maps, C, counts, starts


def _assemble(results, counts, starts):
    out = np.empty((N_TOK, D_OUT), dtype=BF16)
    for e in range(N_EXP):
        n_e = int(counts[e])
        if n_e:
            out[starts[e]:starts[e] + n_e] = results[e]["yT"][:, :n_e].T
    return out


def kernel(x, weight, bias, ids_sorted):
    in_maps, C, counts, starts = _prepare(x, weight, bias, ids_sorted)
    res = _run_spmd(in_maps, C)
    return _assemble(res.results, counts, starts)


# revision 26
# speedup vs baseline: 1.0343x; 1.0343x over previous
"""MoE grouped linear (DMoELinear) on 8 Trainium2 NeuronCores.

Expert-parallel sharding: tokens are sorted by expert id, so expert e's
tokens form one contiguous slice. Core e receives expert e's tokens
(padded to a uniform capacity C = max group size, so all cores run one
SPMD NEFF), expert e's weight and bias, and computes
    yT_e = (x_e @ W_e.T).T.bf16 + b_e.bf16
with the weight block as the stationary matmul operand and tokens as
the moving free dim.

v9 schedule (trace-driven). Measured constraints: each dma_start costs
~0.6us of issuing-engine time; each engine rotates 4 DMA queues (issue
N+4 blocks on N's completion); per-ring throughput under 8-core HBM
contention is ~175-230GB/s. The 3.2MB trickle stream (x + w0/w1) is
the binding constraint for the first ~17us, so it rides few, large
transfers interleaved by first-use deadline; w2 follows split in
halves (db2 runs k-major so it only needs w2k0 at the trickle's
drain), then w3 and the 2-db packs with multi-us slack. (int8 weight
shipping + on-chip upconvert was tried and abandoned: the only fast
int8->bf16 path is ACTIVATE-Copy on the ACT engine (~115G elem/s; the
DVE/Pool tensor_scalar path measures ~9G under load) and it cannot
pace the trickle, while the late tensors it can pace were never
deadline-bound.)

PE schedule: 32 warm matmuls flip the HAM clock gate while the first
tiles land (the real stream gates the start at ~11.5 anyway); db0/db1
interleave k-major (trickle, DMA-paced); db2 runs k-major; dbs 3..14
run chunk-major (c0's whole k-loop first, evicted while c1/c2 —
interleaved so the narrow chunk's LDWEIGHTS hide under 512-wide
drains — still compute) so PSUM banks retire mid-db; db15 splits its
middle 512 region into 256+128+128 pieces with per-piece PSUM tiles
(shared psum tiles serialize readers in the dep tracker) so after the
last matmul only two parallel 128-wide evictions and one 64KB DMA
remain. Bias adds fuse into the PSUM evictions (ACT/DVE alternating).
"""

import numpy as np
import ml_dtypes

N_TOK, D_IN, D_OUT, N_EXP = 8192, 1024, 2048, 8
N_CORES = 8
P = 128
NFREE = 512  # max matmul moving free dim (one PSUM bank of f32)

BF16 = ml_dtypes.bfloat16

_nc_cache: dict[int, object] = {}


def _chunks(C):
    out = []
    off = 0
    while off < C:
        cw = min(NFREE, C - off)
        out.append((off, cw))
        off += cw
    return out


def _build_bass(C: int):
    """Emit the per-core Bass/Tile kernel for token capacity C."""
    import concourse.bass as bass  # noqa: F401  (registers engines)
    import concourse.mybir as mybir
    import concourse.tile as tile
    from concourse import bacc

    dt = mybir.dt
    KT = D_IN // P      # 8 contraction tiles
    DB = D_OUT // P     # 16 output-row blocks
    KW = KT * P         # columns per db block in the flat weight (1024)
    chunks = _chunks(C)
    chunk_of_db = {db: chunks for db in range(DB)}

    nc = bacc.Bacc("TRN2", target_bir_lowering=False)

    # x partition-flat: row p, col ki*C + c  =  x[token c, ki*128+p]
    xf_d = nc.dram_tensor("xf", [P, KT * C], dt.bfloat16, kind="ExternalInput")
    # flat weights: row p, col db*1024 + kt*128 + d  (lhsT slices are
    # contiguous 128-col blocks; multi-db packs are contiguous too).
    wf_d = nc.dram_tensor("wf", [P, DB * KW], dt.bfloat16, kind="ExternalInput")
    bias_d = nc.dram_tensor("biasp", [P, DB], dt.float32, kind="ExternalInput")
    y_d = nc.dram_tensor("yT", [D_OUT, C], dt.bfloat16, kind="ExternalOutput")

    with tile.TileContext(nc) as tc:
        with (
            tc.tile_pool(name="persist", bufs=1) as ppool,
            tc.tile_pool(name="yout", bufs=4) as ypool,
            tc.tile_pool(name="psum", bufs=8, space="PSUM") as pspool,
        ):
            x_tiles = [
                ppool.tile([P, C], dt.bfloat16, name=f"x{ki}", tag=f"x{ki}")
                for ki in range(KT)
            ]

            def x_sl(ki, off, cw):
                return x_tiles[ki][:, off:off + cw]

            w_s = [
                ppool.tile([P, KW], dt.bfloat16, name=f"w{db}", tag=f"w{db}")
                for db in range(4)
            ]
            packs = [
                ppool.tile([P, 2 * KW], dt.bfloat16, name=f"wp{g}", tag=f"wp{g}")
                for g in range(2, 8)
            ]
            bt = ppool.tile([P, DB], dt.float32, name="bias", tag="bias")

            def lhsT(db, ki):
                if db < 4:
                    return w_s[db][:, ki * P:(ki + 1) * P]
                g = db // 2
                off = (db - 2 * g) * KW + ki * P
                return packs[g - 2][:, off:off + P]

            # ── DMA schedule: two HWDGE rings ────────────────────────
            A, B = nc.sync, nc.scalar

            def xdma(ki, eng):
                eng.dma_start(x_tiles[ki][:], xf_d[:, ki * C:(ki + 1) * C])

            # w0/w1 as whole 512KB slabs: splitting them into k-halves
            # starts the PE ~1.5us earlier but only trades idle for
            # mid-trickle stalls (the ramp is ring-bandwidth-bound) and
            # the extra small transfers make the stream more sensitive
            # to the 4-queue rotation — measured worse on bad draws.
            xdma(1, A)
            xdma(0, B)
            A.dma_start(w_s[0][:], wf_d[:, 0:KW])
            B.dma_start(w_s[1][:], wf_d[:, KW:2 * KW])
            xdma(3, A)
            xdma(2, B)
            xdma(5, A)
            xdma(4, B)
            xdma(7, A)
            xdma(6, B)
            B.dma_start(bt[:], bias_d[:])
            A.dma_start(w_s[2][:, 0:4 * P], wf_d[:, 2 * KW:2 * KW + 4 * P])
            A.dma_start(w_s[2][:, 4 * P:8 * P], wf_d[:, 2 * KW + 4 * P:3 * KW])
            B.dma_start(w_s[3][:], wf_d[:, 3 * KW:4 * KW])
            for g in range(2, 8):
                eng = B if g % 2 == 0 else A
                eng.dma_start(packs[g - 2][:], wf_d[:, 2 * g * KW:(2 * g + 2) * KW])

            # ── PE warmup: flip the HAM clock gate (~3.5us of activity)
            # while the first DMAs land. The first real matmul is gated
            # by the w0 slab landing ~11.5 anyway.
            warm = ppool.tile([P, P], dt.bfloat16, name="warm", tag="warm")
            nc.vector.memset(warm[:], 0.0)
            wps = pspool.tile([P, P], dt.float32, name="wps", tag="ps")

            def warm_mm(n):
                for _ in range(n):
                    nc.tensor.matmul(wps[:], warm[:], warm[:], start=True, stop=True)

            warm_mm(32)

            all_psums = {}

            def alloc_chunk(db, j):
                _, cw = chunk_of_db[db][j]
                return pspool.tile([P, cw], dt.float32, name=f"ps{db}_{j}", tag="ps")

            def alloc_psums(db, chunks_j=None):
                js = chunks_j or range(len(chunk_of_db[db]))
                cur = all_psums.setdefault(db, {})
                for j in js:
                    cur[j] = alloc_chunk(db, j)

            def emit_mm(db, ki, j):
                off, cw = chunk_of_db[db][j]
                nc.tensor.matmul(
                    all_psums[db][j][:, :cw],
                    lhsT(db, ki),
                    x_sl(ki, off, cw),
                    start=(ki == 0),
                    stop=(ki == KT - 1),
                )

            def emit_mms(db, ki, chunks_j=None):
                for j in chunks_j or range(len(chunk_of_db[db])):
                    emit_mm(db, ki, j)

            ep = 0
            ysbs = {}

            def new_ysb(db):
                ysbs[db] = ypool.tile([P, C], dt.bfloat16, name="ysb", tag="ysb")
                return ysbs[db]

            def evict_chunk(db, j, ysb):
                nonlocal ep
                off, cw = chunk_of_db[db][j]
                bias_col = bt[:, db:db + 1]
                if ep % 2 == 0:
                    nc.scalar.add(ysb[:, off:off + cw], all_psums[db][j][:, :cw], bias_col)
                else:
                    nc.vector.tensor_scalar_add(
                        ysb[:, off:off + cw], all_psums[db][j][:, :cw], bias_col
                    )
                ep += 1

            def evict(db):
                ysb = new_ysb(db)
                for j in range(len(chunk_of_db[db])):
                    evict_chunk(db, j, ysb)
                return ysb

            def ydma(db, ysb):
                eng = nc.sync if db % 2 == 0 else nc.scalar
                eng.dma_start(y_d[db * P:(db + 1) * P, :], ysb[:])

            # ── Trickle phase ────────────────────────────────────────
            # db0/db1 interleaved by k-step; db1 one step behind so
            # db0's k7 chunks finish (and their PSUM banks evict) while
            # db1's tail runs.
            STAG = 1
            alloc_psums(0)
            alloc_psums(1)
            for step in range(KT + STAG):
                if step < KT:
                    emit_mms(0, step)
                if step >= STAG:
                    emit_mms(1, step - STAG)
            ydma(0, evict(0))
            ydma(1, evict(1))

            # db2 stays k-major: it starts right at the trickle's drain
            # and k-major only needs w2k0 by then (chunk-major would
            # need all of w2, which is still streaming).
            alloc_psums(2)
            for ki in range(KT):
                emit_mms(2, ki)
            ydma(2, evict(2))

            # ── dbs 3..14: chunk-major ───────────────────────────────
            # c0 runs its whole k-loop first and evicts while c1/c2
            # (interleaved so the narrow chunk's LDWEIGHTS hide under
            # the 512-wide drains) are still computing. PSUM banks
            # retire mid-db instead of piling up at db boundaries.
            for db in range(3, DB - 1):
                alloc_psums(db)
                ncks = len(chunk_of_db[db])
                for ki in range(KT):
                    emit_mm(db, ki, 0)
                ysb = new_ysb(db)
                evict_chunk(db, 0, ysb)
                for ki in range(KT):
                    for j in range(1, ncks):
                        emit_mm(db, ki, j)
                for j in range(1, ncks):
                    evict_chunk(db, j, ysb)
                if db == DB - 2:
                    # per-chunk DMAs on both rings so the tail pipelines
                    for j, (off, cw) in enumerate(chunk_of_db[db]):
                        eng = nc.sync if (db + j) % 2 == 0 else nc.scalar
                        eng.dma_start(
                            y_d[db * P:(db + 1) * P, off:off + cw],
                            ysb[:, off:off + cw],
                        )
                else:
                    ydma(db, ysb)

            # ── db15: ordered so the drain is minimal ────────────────
            # (c0, tail-chunk) interleaved first — both evicted and
            # DMA'd while the middle 512 region computes as 256+128+128
            # pieces with per-piece PSUM tiles. After the last matmul
            # only two parallel 128-wide evictions and one 64KB DMA on
            # the sync ring remain (c1a's 256KB rides the scalar ring
            # so the final piece doesn't queue behind it).
            db = DB - 1
            cks = chunk_of_db[db]
            row0 = db * P
            bias_col = bt[:, db:db + 1]
            if len(cks) == 3 and cks[1][1] == NFREE:
                (o0, cw0), (o1, cw1), (o2, cw2) = cks
                h = cw1 // 2
                hh = h // 2
                sub = [(o0, cw0), (o2, cw2), (o1, h),
                       (o1 + h, hh), (o1 + h + hh, cw1 - h - hh)]
                ps = {j: pspool.tile([P, cw], dt.float32, name=f"ps15_{j}", tag="ps")
                      for j, (off, cw) in enumerate(sub)}
                ysb = new_ysb(db)

                def mm15(j, ki):
                    off, cw = sub[j]
                    nc.tensor.matmul(
                        ps[j][:, :cw], lhsT(db, ki), x_sl(ki, off, cw),
                        start=(ki == 0), stop=(ki == KT - 1),
                    )

                for ki in range(KT):
                    mm15(0, ki)
                    mm15(1, ki)
                nc.scalar.add(ysb[:, o0:o0 + cw0], ps[0][:, :cw0], bias_col)
                nc.vector.tensor_scalar_add(
                    ysb[:, o2:o2 + cw2], ps[1][:, :cw2], bias_col
                )
                nc.sync.dma_start(y_d[row0:row0 + P, o0:o0 + cw0],
                                  ysb[:, o0:o0 + cw0])
                nc.scalar.dma_start(y_d[row0:row0 + P, o2:o2 + cw2],
                                    ysb[:, o2:o2 + cw2])
                for ki in range(KT):
                    mm15(2, ki)
                nc.vector.tensor_scalar_add(
                    ysb[:, o1:o1 + h], ps[2][:, :h], bias_col
                )
                nc.scalar.dma_start(y_d[row0:row0 + P, o1:o1 + h],
                                    ysb[:, o1:o1 + h])
                for ki in range(KT):
                    mm15(3, ki)
                    mm15(4, ki)
                o3, cw3 = sub[3]
                o4, cw4 = sub[4]
                nc.scalar.add(ysb[:, o3:o3 + cw3], ps[3][:, :cw3], bias_col)
                nc.vector.tensor_scalar_add(
                    ysb[:, o4:o4 + cw4], ps[4][:, :cw4], bias_col
                )
                nc.sync.dma_start(y_d[row0:row0 + P, o3:o3 + cw3 + cw4],
                                  ysb[:, o3:o3 + cw3 + cw4])
            else:
                # generic fallback (different C): plain chunk-major
                alloc_psums(db)
                for ki in range(KT):
                    emit_mm(db, ki, 0)
                ysb = new_ysb(db)
                evict_chunk(db, 0, ysb)
                for ki in range(KT):
                    for j in range(1, len(cks)):
                        emit_mm(db, ki, j)
                for j in range(1, len(cks)):
                    evict_chunk(db, j, ysb)
                for j, (off, cw) in enumerate(cks):
                    eng = nc.sync if j % 2 == 0 else nc.scalar
                    eng.dma_start(
                        y_d[row0:row0 + P, off:off + cw], ysb[:, off:off + cw]
                    )

    nc.compile()
    return nc


def _run_spmd(in_maps, C, trace=False, trace_cores=None):
    from concourse.bass_utils import run_bass_kernel_spmd

    nc = _nc_cache.get(C)
    if nc is None:
        nc = _build_bass(C)
        _nc_cache[C] = nc
    return run_bass_kernel_spmd(
        nc,
        in_maps,
        core_ids=list(range(N_CORES)),
        trace=trace,
        trace_cores=trace_cores,
    )


def _prepare(x, weight, bias, ids_sorted):
    """Host-side routing: returns (in_maps, C, counts, starts)."""
    x = np.asarray(x)
    weight = np.asarray(weight)
    bias = np.asarray(bias)
    ids = np.asarray(ids_sorted)

    counts = np.bincount(ids, minlength=N_EXP).astype(np.int64)
    starts = np.zeros(N_EXP, dtype=np.int64)
    starts[1:] = np.cumsum(counts)[:-1]
    C = max(int(counts.max()), 2)
    C += C % 2

    KT = D_IN // P
    DB = D_OUT // P
    xb = x.astype(BF16)
    in_maps = []
    for e in range(N_EXP):
        n_e = int(counts[e])
        xeT = np.zeros((D_IN, C), dtype=BF16)
        if n_e:
            xeT[:, :n_e] = xb[starts[e]:starts[e] + n_e].T
        # partition-flat x: row p, col ki*C + c = x[token c, ki*128+p]
        xf = np.ascontiguousarray(
            xeT.reshape(KT, P, C).transpose(1, 0, 2)
        ).reshape(P, KT * C)
        # flat weight: row p, col db*1024 + kt*128 + d  = W_e[db*128+d, kt*128+p]
        weT = weight[e].T.astype(BF16)  # [d_in, d_out]
        wf = np.ascontiguousarray(
            weT.reshape(KT, P, DB, P).transpose(1, 2, 0, 3)
        ).reshape(P, DB * KT * P)
        bp = np.ascontiguousarray(
            bias[e].astype(BF16).astype(np.float32).reshape(DB, P).T
        )
        in_maps.append({"xf": xf, "wf": wf, "biasp": bp})
    return in_maps, C, counts, starts


def _assemble(results, counts, starts):
    out = np.empty((N_TOK, D_OUT), dtype=BF16)
    for e in range(N_EXP):
        n_e = int(counts[e])
        if n_e:
            out[starts[e]:starts[e] + n_e] = results[e]["yT"][:, :n_e].T
    return out


def kernel(x, weight, bias, ids_sorted):
    in_maps, C, counts, starts = _prepare(x, weight, bias, ids_sorted)
    res = _run_spmd(in_maps, C)
    return _assemble(res.results, counts, starts)
